# revision 2
# baseline (speedup 1.0000x reference)
"""Trainium2 Bass kernel v5: cache-distance -> exp kernel -> vocab histogram.

Math (per cache row i): kern_i = exp(||cache_h[i] - h_t|| / 0.2)
                        cache_p[v] = sum_{i: word_ids[i]==v} kern_i
                        out = log_softmax(cache_p)[None, :]

Precision: min |log_softmax| ~0.075 makes the 2e-2 rel gate demand bin
errors <~1e-4: y ships f16, and the device accumulates kern - kc
(kc ~ median kern, host-estimated) so the f16 quantization of the
kern-carrying operand shrinks ~5x; host adds kc * bincount(word_ids).

Device strategy (8 cores):
  - y pre-transposed [128, 4, RPC] f16, ONE DMA per 4096-row chunk,
    first chunk's DMA issued before all constants (the v4 startup spent
    ~35us loading constants + one-hots before any compute)
  - squares split ACT (Square, scale=32) / DVE (tensor_tensor, 2x mode);
    PE one-hot-column matmuls reduce each chunk to dist^2 [8,512] PSUM
  - kern' = exp(exp(0.5*ln(s*d2))) - kc via ACT chain + biased PSUM copy
  - histogram: host sorts by word_id globally, deals 1024-element runs
    round-robin to cores (batch B covers the same vocab window
    [n0, n0+3] on every core); host uploads the pure m-one-hot O (fp8,
    [128,128] per batch, streamed per chunk). Per chunk DVE builds ALL
    32 batches' [128,4] kern-weighted window one-hots in 2 wide
    tensor_tensor ops (stride-0 broadcasts); per batch one PE matmul
    lhsT=O-slice fp8 x rhs=Bwk-slice f16 -> [128 m, 4 n] accumulated at
    PSUM free-offset n0 into histT [128, 512]. Batches alternate between
    two PSUM hist banks so back-to-back accumulation drains overlap;
    host sums both.
Host: histT[m, n] -> vocab order, + kc*count, sum partials, log_softmax.
"""

import os
import sys

for _p in ("/root/.axon_site", "/root/.axon_site/_ro/trn_rl_repo",
           "/root/.axon_site/_ro/pypackages"):
    if os.path.isdir(_p) and _p not in sys.path:
        sys.path.append(_p)

import numpy as np

VOCAB = 50257
N_CACHE = 262144
D = 512
SMOOTH = 0.2
NCORES = 8
RPC = N_CACHE // NCORES        # 32768 rows per core
NCHUNK = 8
CHUNK = RPC // NCHUNK          # 4096 rows per chunk
GPC = CHUNK // 512             # 8 groups of 512 rows per chunk
BPC = RPC // 128               # 256 batches of 128 elements per core
NGB = N_CACHE // 1024          # 256 global sorted batches (1024 elems each)
WIN = 4                        # vocab n-window (wid//128 - n0) per batch
NVT = 512                      # histT free dim (n up to 393, padded)

ACT_SCALE = 32.0               # ACT squares are (32*y)^2 = 1024*y^2; the
                               # DVE-squared blocks get lhsT=1024 in the
                               # reduce instead, so dist PSUM is uniformly
                               # 1024*dist^2 and Ln uses 25/1024

_CACHE = {}


def _patch_act_tables():
    """Restrict the activation table-set chooser to
    natural_log_exp_and_others (covers square/ln/exp/copy) so the whole
    kernel needs exactly one ACT_TABLE_LOAD instead of alternating between
    the ln-only and exp-only sets every chunk (~1.3us per reload)."""
    import concourse.hw_specs as hw_specs
    import concourse.bacc as bacc

    if getattr(hw_specs.get_activation_tables, "_histkernel_patched", False):
        return
    orig = hw_specs.get_activation_tables

    def patched(module_arch):
        tabs = orig(module_arch)
        return {
            name: (fns if name == "natural_log_exp_and_others" else set())
            for name, fns in tabs.items()
        }

    patched._histkernel_patched = True
    hw_specs.get_activation_tables = patched
    bacc.get_activation_tables = patched


def _build_program(n0, kc):
    import concourse.bacc as bacc
    import concourse.tile as tile
    import concourse.mybir as mybir

    _patch_act_tables()

    f32, f16, f8 = mybir.dt.float32, mybir.dt.float16, mybir.dt.float8e4
    AF = mybir.ActivationFunctionType
    ALU = mybir.AluOpType

    nc = bacc.Bacc("TRN2", target_bir_lowering=False, debug=False,
                   num_devices=NCORES)

    xt_d = nc.dram_tensor("xt", [128, NCHUNK, 4 * CHUNK], f16,
                          kind="ExternalInput")
    oo_d = nc.dram_tensor("oo", [128, BPC * 128], f8, kind="ExternalInput")
    bn_d = nc.dram_tensor("bn", [128, BPC, WIN], f16, kind="ExternalInput")
    oh_d = nc.dram_tensor("oh", [128, 128], f16, kind="ExternalInput")
    id_d = nc.dram_tensor("idm", [8, 8], f32, kind="ExternalInput")
    hist_d = nc.dram_tensor("hist", [128, 2 * NVT], f32,
                            kind="ExternalOutput")

    with tile.TileContext(nc) as tc:
        with (
            tc.tile_pool(name="const", bufs=1) as cpool,
            tc.tile_pool(name="x", bufs=3) as xpool,
            tc.tile_pool(name="sq", bufs=6) as sqpool,
            tc.tile_pool(name="s", bufs=3) as spool,
            tc.tile_pool(name="kt", bufs=3) as ktpool,
            tc.tile_pool(name="oo", bufs=3) as oopool,
            tc.tile_pool(name="bw", bufs=8) as bwpool,
            tc.tile_pool(name="out", bufs=1) as opool,
            tc.tile_pool(name="pshist", bufs=2, space="PSUM") as pshist,
            tc.tile_pool(name="psdist", bufs=3, space="PSUM") as psdist,
            tc.tile_pool(name="pskt", bufs=1, space="PSUM") as pskt,
        ):
            xt_ap = xt_d.ap()
            oo_ap = oo_d.ap()

            def emit_load(ch):
                # two DMAs per chunk: 16KB-contiguous per-partition
                # descriptors, half-chunk dependency granularity
                xa = xpool.tile([128, 2, CHUNK], f16)
                nc.sync.dma_start(xa[:], xt_ap[:, ch, 0:2 * CHUNK])
                xb = xpool.tile([128, 2, CHUNK], f16)
                nc.sync.dma_start(xb[:], xt_ap[:, ch, 2 * CHUNK:4 * CHUNK])
                return (xa, xb)

            def emit_load_oo(chpair):
                # one-hot matrices for a PAIR of chunks (8KB descriptors)
                oo = oopool.tile([128, 2, 32 * 128], f8)
                nc.sync.dma_start(
                    oo[:], oo_ap[:, chpair * 8192:(chpair + 1) * 8192])
                return oo

            # first chunk of y starts moving before anything else
            x = emit_load(0)
            oh = cpool.tile([128, 128], f16)
            nc.sync.dma_start(oh[:], oh_d.ap())
            idm = cpool.tile([8, 8], f32)
            nc.sync.dma_start(idm[:], id_d.ap())
            oo0 = emit_load_oo(0)
            bn = cpool.tile([128, BPC, WIN], f16)
            nc.sync.dma_start(bn[:], bn_d.ap())

            hist_a = pshist.tile([128, NVT], f32)
            hist_b = pshist.tile([128, NVT], f32)
            hists = [hist_a, hist_b]

            def emit_sq(ch, x):
                # blocks 0-1 on ACT (pre-scaled), blocks 2-3 on DVE
                # (unscaled; their reduce lhsT carries the 1024)
                sqs = []
                for c in range(4):
                    xs = x[c // 2][:, c % 2, :]
                    sq = sqpool.tile([128, CHUNK], f16)
                    if c < 2:
                        nc.scalar.activation(sq[:], xs, AF.Square,
                                             scale=ACT_SCALE)
                    else:
                        nc.vector.tensor_tensor(sq[:], xs, xs, ALU.mult)
                    sqs.append(sq)
                return sqs

            def emit_reduce(ch, sqs):
                dist = psdist.tile([8, 512], f32)
                mm = 0
                for c in (2, 3, 0, 1):        # DVE-squared blocks first
                    base = 0 if c < 2 else 64   # ones vs 1024-valued cols
                    for g in range(GPC):
                        nc.tensor.matmul(
                            dist[:],
                            oh[:, base + g * 8:base + (g + 1) * 8],
                            sqs[c][:, g * 512:(g + 1) * 512],
                            start=(mm == 0),
                            stop=(mm == 4 * GPC - 1),
                        )
                        mm += 1
                return dist

            def emit_kern(ch, dist):
                lnscale = 25.0 / (ACT_SCALE * ACT_SCALE)
                lg = spool.tile([8, 512], f32)
                nc.scalar.activation(lg[:], dist[:], AF.Ln, scale=lnscale)
                d5 = spool.tile([8, 512], f32)
                nc.scalar.activation(d5[:], lg[:], AF.Exp, scale=0.5)
                kern = spool.tile([8, 512], f32)
                nc.scalar.activation(kern[:], d5[:], AF.Exp)
                return kern

            def emit_transp(kern):
                ktp = pskt.tile([128, 32], f32)
                for c4 in range(4):
                    nc.tensor.transpose(
                        ktp[:, c4 * 8:(c4 + 1) * 8],
                        kern[:, c4 * 128:(c4 + 1) * 128],
                        idm[:],
                    )
                kt = ktpool.tile([128, 32], f16)
                # kern' = kern - kc: range-reduce before the f16 hist path
                nc.vector.tensor_scalar(kt[:], ktp[:], -kc, None, ALU.add)
                return kt

            def emit_bwk(ch, kt):
                # all 32 batches' [128, 4] window one-hots in two wide DVE
                # ops. bn holds (wid//128 - n0[B]) - w at [p, B, w], so the
                # window one-hot is is_equal(bn-slice, 0); then multiply by
                # kern' broadcast (stride-0) along the window dim.
                onh = bwpool.tile([128, 32, WIN], f16)
                nc.vector.tensor_scalar(
                    onh[:], bn[:, ch * 32:(ch + 1) * 32, :], 0.0, None,
                    ALU.is_equal,
                )
                bwk = bwpool.tile([128, 32, WIN], f16)
                ktb = kt[:].unsqueeze(2).broadcast_to([128, 32, WIN])
                nc.vector.tensor_tensor(bwk[:], onh[:], ktb, ALU.mult)
                return bwk

            def emit_hist(ch, oo, bwk):
                for bl in range(32):
                    B = ch * 32 + bl
                    nc.tensor.matmul(
                        hists[bl % 2][:, n0[B]:n0[B] + WIN],
                        oo[:, ch % 2, bl * 128:(bl + 1) * 128],
                        bwk[:, bl, :],
                        start=False, stop=True, skip_group_check=True,
                    )

            # 1-chunk-staggered pipeline
            pend = None
            cur_oo = oo0
            zero_done = False
            for ch in range(NCHUNK):
                nxt = emit_load(ch + 1) if ch + 1 < NCHUNK else None
                nxt_oo = emit_load_oo((ch + 1) // 2) \
                    if (ch + 1 < NCHUNK and (ch + 1) % 2 == 0) else None
                sqs = emit_sq(ch, x)
                if pend is not None:
                    kern = emit_kern(pend[0], pend[1])
                dist = emit_reduce(ch, sqs)
                if not zero_done:
                    # zero both hist banks once (lhsT of zeros x anything)
                    zt = bwpool.tile([128, 128], f16)
                    nc.vector.tensor_scalar(zt[:], sqs[0][:, 0:128],
                                            0.0, None, ALU.mult)
                    for h in hists:
                        nc.tensor.matmul(h[:], zt[:], sqs[0][:, 0:NVT],
                                         start=True, stop=False,
                                         skip_group_check=True)
                    zero_done = True
                if pend is not None:
                    kt = emit_transp(kern)
                    bwk = emit_bwk(pend[0], kt)
                    emit_hist(pend[0], pend[2], bwk)
                pend = (ch, dist, cur_oo)
                if nxt_oo is not None:
                    cur_oo = nxt_oo
                if nxt is not None:
                    x = nxt
            kern = emit_kern(pend[0], pend[1])
            kt = emit_transp(kern)
            bwk = emit_bwk(pend[0], kt)
            emit_hist(pend[0], pend[2], bwk)

            hist_sb = opool.tile([128, 2 * NVT], f32)
            nc.scalar.copy(hist_sb[:, 0:NVT], hists[0][:])
            nc.scalar.copy(hist_sb[:, NVT:2 * NVT], hists[1][:])
            nc.sync.dma_start(hist_d.ap(), hist_sb[:])

    nc.compile()
    return nc


def _prep_inputs(h_t, cache_h, word_ids):
    import ml_dtypes

    h_t = np.asarray(h_t, dtype=np.float32)
    cache_h = np.ascontiguousarray(np.asarray(cache_h, dtype=np.float32))
    word_ids = np.asarray(word_ids, dtype=np.int64)

    order = np.argsort(word_ids, kind="stable")
    wsorted = word_ids[order]
    wg = wsorted.reshape(NGB, 1024)
    ng = wg // 128
    n0 = ng.min(1)
    nmax = ng.max(1)
    assert (nmax - n0).max() <= WIN - 1, \
        f"vocab window overflow: {(nmax - n0).max()}"

    deal = order.reshape(NGB, 128, NCORES)     # [B, p, core]
    wdeal = wsorted.reshape(NGB, 128, NCORES)  # [B, p, core]

    B_ = np.arange(NGB)
    bl = B_ % 32
    rowbase = (B_ // 32) * CHUNK + (bl % 8) * 512 + (bl // 8) * 128
    ridx = rowbase[:, None] + np.arange(128)[None, :]    # [B, p]

    samp = cache_h[:4096].astype(np.float64) - h_t.astype(np.float64)
    dsamp = np.sqrt((samp * samp).sum(1))
    kc = float(np.exp(np.median(dsamp) / SMOOTH))

    y = cache_h - h_t[None, :]
    oh = np.zeros((128, 128), np.float16)
    for g in range(8):
        oh[:, g * 8 + g] = 1.0
        oh[:, 64 + g * 8 + g] = 1024.0
    idm = np.eye(8, dtype=np.float32)
    bcol = (np.arange(BPC)[None, :] * 128).astype(np.int64)
    wrel = np.arange(WIN)[None, None, :]                 # [1, 1, w]

    in_maps = []
    for k in range(NCORES):
        rows = np.empty(RPC, np.int64)
        rows[ridx.ravel()] = deal[:, :, k].ravel()
        # layout [128, NCHUNK, 4*CHUNK]: partition p, chunk ch holds the
        # four D-blocks of that chunk contiguously (one 32KB run per
        # partition per chunk-DMA)
        yk = np.ascontiguousarray(
            y[rows].T.reshape(4, 128, NCHUNK, CHUNK)
            .transpose(1, 2, 0, 3).reshape(128, NCHUNK, 4 * CHUNK)
        ).astype(np.float16)
        mk = (wdeal[:, :, k] % 128).T            # [p, B]
        oo = np.zeros((128, BPC * 128), ml_dtypes.float8_e4m3)
        oo[np.arange(128)[:, None], bcol + mk] = 1.0
        # bn[p, B*4 + w] = (wid//128 - n0[B]) - w  (one-hot via == 0)
        nrel = (wdeal[:, :, k] // 128 - n0[:, None])     # [B, p]
        bn = np.ascontiguousarray(
            (nrel.T[:, :, None] - wrel)).astype(np.float16)   # [p, B, w]
        in_maps.append({
            "xt": yk, "oo": oo, "bn": bn, "oh": oh, "idm": idm,
        })
    return in_maps, n0, kc


def kernel(h_t, cache_h, word_ids):
    from concourse.bass_utils import run_bass_kernel_spmd

    in_maps, n0, kc = _prep_inputs(h_t, cache_h, word_ids)
    key = hash(np.asarray(word_ids, np.int64).tobytes()
               + np.asarray(h_t, np.float32).tobytes())
    if _CACHE.get("key") != key:
        _CACHE["nc"] = _build_program([int(v) for v in n0], kc)
        _CACHE["key"] = key
    nc = _CACHE["nc"]

    res = run_bass_kernel_spmd(nc, in_maps, list(range(NCORES)))

    hist = np.zeros((128, NVT), np.float64)
    for k in range(NCORES):
        h = res.results[k]["hist"].astype(np.float64)
        hist += h[:, 0:NVT] + h[:, NVT:2 * NVT]
    # histT[m, n] holds vocab v = n*128 + m (device stored kern - kc)
    cache_p = hist.T.ravel()[:VOCAB]
    count = np.bincount(np.asarray(word_ids, np.int64), minlength=VOCAB)
    cache_p = cache_p + kc * count.astype(np.float64)

    m = cache_p.max()
    lse = m + np.log(np.exp(cache_p - m).sum())
    out = (cache_p - lse).astype(np.float32)
    return out[None, :]
